# revision 30
# baseline (speedup 1.0000x reference)
"""ConvLSTM cell (B=32, C_IN=32, HC=64, H=W=64, K=3) on 8 trn2 NeuronCores.

Strategy: data-parallel over batch (4 images per core), weights replicated.
The 3x3 conv runs as Winograd F(2,3) along y: the 3 dy-taps become 4
transformed points xi over 2-row output tiles, cutting PE streaming columns
1.5x (12 matmul passes x half-pixels vs 18 x full-pixels per image).
x (32ch) and h (64ch) are concatenated and y-zero-padded on host into one
bf16 tensor [4, 96, 66, 64]; each 16-row block loads a contiguous
[96, 18, 64] tile (pad rows absorb the y borders; x borders use restricted
column ranges, center dx first with start=True).
The input transform V = B^T d (4 strided adds, bf16) is split across
vector/gpsimd; the output transform z = A^T m (4 f32 adds per chunk,
reading PSUM) runs on vector for chunk0 ([f,i]) and gpsimd for chunk1
([o,g]). LSTM elementwise math and the merged [h_new | c_new] output DMA
follow the same deferred-tail pipeline as the direct kernel.
"""

import os
import sys

import numpy as np

if "/opt/trn_rl_repo" not in sys.path:
    sys.path.insert(0, "/opt/trn_rl_repo")

import ml_dtypes

BF16 = ml_dtypes.bfloat16

B, C_IN, HC, H, W, K = 32, 32, 64, 64, 64, 3
N_CORES = 8
B_LOC = B // N_CORES  # 4 images per core
CTOT = C_IN + HC  # 96 combined input channels
RPB = 16  # output rows per block (last image: 8 for a shorter tail)
# dx tap order: center (dx=1) first so start=True covers every element
DX_ORDER = [1, 0, 2]
# per-dx border-valid output/input column ranges: (cout0, cin0, ncols)
DX_COLS = {0: (1, 0, 63), 1: (0, 0, 64), 2: (0, 1, 63)}

_CACHE: dict = {}


def _build_program():
    import concourse.bacc as bacc
    import concourse.mybir as mybir
    import concourse.tile as tile

    nc = bacc.Bacc("TRN2", target_bir_lowering=False, debug=False)
    f32 = mybir.dt.float32
    bf16 = mybir.dt.bfloat16
    AF = mybir.ActivationFunctionType

    xh_d = nc.dram_tensor("xh", [B_LOC, CTOT, H + 2, W], bf16, kind="ExternalInput").ap()
    c_d = nc.dram_tensor("c", [B_LOC, HC, H, W], bf16, kind="ExternalInput").ap()
    # Winograd-y weights: [c, xi(4), dx(3), chunk(2), m(128)]
    u_d = nc.dram_tensor("u", [CTOT, 4 * 3 * 2 * 128], bf16, kind="ExternalInput").ap()
    b_d = nc.dram_tensor("bias", [128, 2], f32, kind="ExternalInput").ap()
    # out[:, 0] = h_new, out[:, 1] = c_new (bf16; host converts to f32)
    out_d = nc.dram_tensor(
        "out", [B_LOC, 2, HC, H, W], bf16, kind="ExternalOutput"
    ).ap()

    with tile.TileContext(nc) as tc:
        with (
            tc.tile_pool(name="const", bufs=1) as constp,
            tc.tile_pool(name="pt", bufs=4) as ptp,
            tc.tile_pool(name="vt", bufs=4) as vtp,
            tc.tile_pool(name="psum", bufs=2, space="PSUM") as pp,
            tc.tile_pool(name="work", bufs=4) as sp,
        ):
            # PE prewarm first: dummy matmuls on zeroed tiles bridge the HAM
            # clock-gate window while the weight/input DMAs run
            dw = constp.tile([CTOT, 128], bf16)
            nc.vector.memset(dw[:], 0.0)
            drh = constp.tile([CTOT, 512], bf16)
            nc.vector.memset(drh[:], 0.0)

            u_sb = constp.tile([CTOT, 4 * 3 * 2 * 128], bf16)
            nc.scalar.dma_start(u_sb[:], u_d)
            b_sb = constp.tile([128, 2], f32)
            nc.scalar.dma_start(b_sb[:], b_d)

            pwp = pp.tile([128, 4 * 8 * W], f32, tag="P", name="pw")
            for _ in range(14):
                nc.tensor.matmul(
                    pwp[:, 0:512], dw[:], drh[:], start=True, stop=True
                )

            def stage_b(st):
                # deferred tail of a block: tanh(c_new), h_new, output DMA
                b_, y0_, rpb_, so_, chn_, i_ = st
                tch = sp.tile([64, rpb_ * W], bf16, tag="tch", name=f"tch{i_}")
                nc.scalar.activation(tch[:], chn_[64:128, :], AF.Tanh)
                nc.vector.tensor_mul(chn_[0:64, :], so_[:], tch[:])
                hw_ = rpb_ // 2 * W
                nc.sync.dma_start(
                    out_d[b_, :, :, y0_ : y0_ + rpb_ : 2, :].rearrange(
                        "t c y x -> (t c) y x"
                    ),
                    chn_[:, 0:hw_].rearrange("p (y x) -> p y x", x=W),
                )
                nc.sync.dma_start(
                    out_d[b_, :, :, y0_ + 1 : y0_ + rpb_ : 2, :].rearrange(
                        "t c y x -> (t c) y x"
                    ),
                    chn_[:, hw_ : 2 * hw_].rearrange("p (y x) -> p y x", x=W),
                )

            pending = None
            bi = 0
            for b in range(B_LOC):
                rpb = 8 if b == B_LOC - 1 else RPB
                nblk = H // rpb
                for blk in range(nblk):
                    y0 = blk * rpb
                    nt = rpb // 2  # 2-row winograd tiles
                    nrows = rpb + 2
                    blk_px = rpb * W
                    # y-padded input rows y0 .. y0+rpb+1, stored parity-major
                    # (evens then odds) so the B^T transform reads contiguous
                    # row groups; the reorder rides in the DMA access pattern
                    pt = ptp.tile(
                        [CTOT, nrows * W], bf16, tag="ptb", name=f"ptb{bi}"
                    )
                    hr = nrows // 2
                    pt4 = pt[:].rearrange("c (p r x) -> c p r x", p=2, x=W)
                    nc.sync.dma_start(
                        pt[:, 0 : hr * W].rearrange("c (r x) -> c r x", x=W),
                        xh_d[b, :, y0 : y0 + nrows : 2, :],
                    )
                    nc.sync.dma_start(
                        pt[:, hr * W : 2 * hr * W].rearrange("c (r x) -> c r x", x=W),
                        xh_d[b, :, y0 + 1 : y0 + nrows : 2, :],
                    )

                    # input transform V = B^T d  (per tile rows 2t..2t+3)
                    V = vtp.tile([CTOT, 4 * nt * W], bf16, tag="V", name=f"V{bi}")
                    V3 = V[:].rearrange("c (q t x) -> c q t x", q=4, x=W)
                    # input transform on gpsimd: it runs blocks ahead of the
                    # PE (deep pools), so its slowness stays off the
                    # PSUM-evacuation critical path
                    # d_k = local rows k, k+2, ... (parity k%2, offset k//2)
                    d = [
                        pt4[:, k % 2, k // 2 : k // 2 + nt, :] for k in range(4)
                    ]
                    nc.gpsimd.tensor_sub(V3[:, 0], d[0], d[2])
                    nc.gpsimd.tensor_add(V3[:, 1], d[1], d[2])
                    nc.gpsimd.tensor_sub(V3[:, 2], d[2], d[1])
                    nc.gpsimd.tensor_sub(V3[:, 3], d[1], d[3])

                    zs = []
                    for chunk in range(2):
                        P = pp.tile(
                            [128, 4 * nt * W], f32, tag="P", name=f"P{bi}_{chunk}"
                        )
                        P4 = P[:].rearrange("p (q t x) -> p q t x", q=4, x=W)
                        # q=1,2 first: the transform's staging copies (which
                        # need only m1/m2) start while m0/m3 still compute
                        for q in (1, 2, 0, 3):
                            for dx in DX_ORDER:
                                lo = ((q * 3 + dx) * 2 + chunk) * 128
                                cout0, cin0, ncol = DX_COLS[dx]
                                nc.tensor.matmul(
                                    P4[:, q, :, cout0 : cout0 + ncol],
                                    u_sb[:, lo : lo + 128],
                                    V3[:, q, :, cin0 : cin0 + ncol],
                                    start=(dx == 1),
                                    stop=(dx == 2),
                                )
                        # output transform: even rows = m0+m1+m2,
                        # odd rows = m1-m2-m3. GpSimd has no PSUM port and
                        # vector ops may read at most ONE PSUM operand, so
                        # ACT stages m1/m2 to bf16 SBUF; then ta/tb are pure
                        # bf16 on gpsimd and ze/zo are single-PSUM on vector.
                        # z is parity-major: [even output rows | odd rows],
                        # so ze/zo writes are contiguous (stride-2 row writes
                        # halve DVE throughput)
                        z = sp.tile(
                            [128, blk_px], bf16, tag=f"z{chunk}", name=f"z{chunk}_{bi}"
                        )
                        z4 = z[:].rearrange("p (e t x) -> p e t x", e=2, x=W)
                        s1 = sp.tile(
                            [128, nt * W], bf16, tag=f"s1{chunk}", name=f"s1{chunk}_{bi}"
                        )
                        s13 = s1[:].rearrange("p (t x) -> p t x", x=W)
                        nc.scalar.activation(s13, P4[:, 1], AF.Copy)
                        # m2 staged to SBUF; split across ACT/DVE to balance
                        s2 = sp.tile(
                            [128, nt * W], bf16, tag=f"s2{chunk}", name=f"s2{chunk}_{bi}"
                        )
                        s23 = s2[:].rearrange("p (t x) -> p t x", x=W)
                        if chunk == 0:
                            nc.scalar.activation(s23, P4[:, 2], AF.Copy)
                        else:
                            nc.vector.tensor_copy(s23, P4[:, 2])
                        ta = sp.tile(
                            [128, nt * W], bf16, tag=f"ta{chunk}", name=f"ta{chunk}_{bi}"
                        )
                        ta3 = ta[:].rearrange("p (t x) -> p t x", x=W)
                        nc.vector.tensor_add(ta3, s13, s23)
                        nc.vector.tensor_add(z4[:, 0], ta3, P4[:, 0])
                        tb = sp.tile(
                            [128, nt * W], bf16, tag=f"tb{chunk}", name=f"tb{chunk}_{bi}"
                        )
                        tb3 = tb[:].rearrange("p (t x) -> p t x", x=W)
                        nc.vector.tensor_sub(tb3, s13, s23)
                        nc.vector.tensor_sub(z4[:, 1], tb3, P4[:, 3])
                        zs.append(z)

                    z0, z1 = zs
                    # elementwise LSTM math: z0 = [f | i], z1 = [o | g]
                    s_fi = sp.tile([128, blk_px], bf16, tag="sfi", name=f"sfi{bi}")
                    nc.scalar.activation(
                        s_fi[:], z0[:], AF.Sigmoid, bias=b_sb[:, 0:1]
                    )
                    so = sp.tile([64, blk_px], bf16, tag="so", name=f"so{bi}")
                    nc.scalar.activation(
                        so[:], z1[0:64, :], AF.Sigmoid, bias=b_sb[0:64, 1:2]
                    )
                    cg = sp.tile([128, blk_px], bf16, tag="cg", name=f"cg{bi}")
                    nc.scalar.activation(
                        cg[64:128, :], z1[64:128, :], AF.Tanh, bias=b_sb[64:128, 1:2]
                    )
                    hw_ = nt * W
                    nc.gpsimd.dma_start(
                        cg[0:64, 0:hw_].rearrange("c (t x) -> c t x", x=W),
                        c_d[b, :, y0 : y0 + rpb : 2, :],
                    )
                    nc.gpsimd.dma_start(
                        cg[0:64, hw_ : 2 * hw_].rearrange("c (t x) -> c t x", x=W),
                        c_d[b, :, y0 + 1 : y0 + rpb : 2, :],
                    )
                    # prd = [f*c | i*g]
                    prd = sp.tile([128, blk_px], bf16, tag="prd", name=f"prd{bi}")
                    nc.vector.tensor_mul(prd[:], s_fi[:], cg[:])
                    # partition-shift via DMA (sync queue) frees DVE cycles
                    igc = sp.tile([64, blk_px], bf16, tag="igc", name=f"igc{bi}")
                    nc.sync.dma_start(igc[:], prd[64:128, :])
                    # chn = [h_new | c_new] merged output tile
                    chn = sp.tile([128, blk_px], bf16, tag="chn", name=f"chn{bi}")
                    nc.vector.tensor_add(chn[64:128, :], prd[0:64, :], igc[:])
                    if pending is not None:
                        stage_b(pending)
                    pending = (b, y0, rpb, so, chn, bi)
                    bi += 1
            stage_b(pending)

    nc.compile()
    return nc


def get_program():
    if "nc" not in _CACHE:
        _CACHE["nc"] = _build_program()
    return _CACHE["nc"]


def _prep_host(inputs):
    """Pack Winograd weights/biases; pad+convert x/h to bf16; per-core maps."""
    x = np.asarray(inputs["x"], np.float32)
    h = np.asarray(inputs["hidden_state"], np.float32)
    c = np.ascontiguousarray(np.asarray(inputs["cell_state"], np.float32)).astype(BF16)

    # gate order [f, i, o, g] -> chunk0=[f,i], chunk1=[o,g]
    gx = [inputs["w_xf"], inputs["w_xi"], inputs["w_xo"], inputs["w_xg"]]
    gh = [inputs["w_hf"], inputs["w_hi"], inputs["w_ho"], inputs["w_hg"]]
    wx = np.stack([np.asarray(a, np.float32) for a in gx])  # [4, HC, C_IN, 3, 3]
    wh = np.stack([np.asarray(a, np.float32) for a in gh])  # [4, HC, HC, 3, 3]
    Wc = np.concatenate([wx, wh], axis=2)  # [4, HC, 96, 3, 3] (g, o, c, dy, dx)
    G = np.array(
        [[1, 0, 0], [0.5, 0.5, 0.5], [0.5, -0.5, 0.5], [0, 0, 1]], np.float32
    )
    # U[c, xi, dx, g, o] = sum_dy G[xi, dy] * Wc[g, o, c, dy, dx]
    U = np.einsum("xd,gocdk->cxkgo", G, Wc)
    U = np.ascontiguousarray(U).reshape(CTOT, 4 * 3 * 2 * 128).astype(BF16)

    bf = np.asarray(inputs["b_xf"], np.float32) + np.asarray(inputs["b_hf"], np.float32)
    bi = np.asarray(inputs["b_xi"], np.float32) + np.asarray(inputs["b_hi"], np.float32)
    bo = np.asarray(inputs["b_xo"], np.float32) + np.asarray(inputs["b_ho"], np.float32)
    bg = np.asarray(inputs["b_xg"], np.float32) + np.asarray(inputs["b_hg"], np.float32)
    bias = np.stack(
        [np.concatenate([bf, bi]), np.concatenate([bo, bg])], axis=1
    ).astype(np.float32)  # [128, 2]

    xh = np.zeros((B, CTOT, H + 2, W), BF16)
    xh[:, :, 1 : H + 1] = np.concatenate([x, h], axis=1).astype(BF16)

    in_maps = []
    for i in range(N_CORES):
        s = slice(i * B_LOC, (i + 1) * B_LOC)
        in_maps.append(
            {
                "xh": xh[s],
                "c": c[s],
                "u": U,
                "bias": bias,
            }
        )
    return in_maps


def run(inputs, trace=False, trace_kwargs=None):
    from concourse.bass_utils import run_bass_kernel_spmd

    nc = get_program()
    in_maps = _prep_host(inputs)
    res = run_bass_kernel_spmd(
        nc,
        in_maps,
        list(range(N_CORES)),
        trace=trace,
        **(trace_kwargs or {}),
    )
    h_new = np.concatenate(
        [np.asarray(r["out"][:, 0]).astype(np.float32) for r in res.results], 0
    )
    c_new = np.concatenate(
        [np.asarray(r["out"][:, 1]).astype(np.float32) for r in res.results], 0
    )
    return (h_new, c_new), res


def kernel(**inputs):
    (h_new, c_new), _ = run(inputs, trace=False)
    return (h_new, c_new)


# revision 32
# speedup vs baseline: 1.4600x; 1.4600x over previous
"""ConvLSTM cell (B=32, C_IN=32, HC=64, H=W=64, K=3) on 8 trn2 NeuronCores.

Strategy: data-parallel over batch (4 images per core), weights replicated.
x (32ch) and h (64ch) are concatenated on host into one bf16 tensor; each
16-row block loads a contiguous [96, 18, 64] tile (1-row halo each side).
The fused conv (-> 256 gate channels) is 9 shifted matmuls per 128-channel
chunk accumulating in PSUM; image-border taps use row/column-restricted
access patterns instead of a zero-padded layout (the center tap runs first
with start=True so every PSUM element is initialized).
Gate chunks: chunk0 = [f, i], chunk1 = [o, g]; the LSTM elementwise math
runs mostly full-width, with one gpsimd partition-shift copy and a single
merged [h_new | c_new] output DMA per block.
"""

import os
import sys

import numpy as np

if "/opt/trn_rl_repo" not in sys.path:
    sys.path.insert(0, "/opt/trn_rl_repo")

import ml_dtypes

BF16 = ml_dtypes.bfloat16

B, C_IN, HC, H, W, K = 32, 32, 64, 64, 64, 3
N_CORES = 8
B_LOC = B // N_CORES  # 4 images per core
CTOT = C_IN + HC  # 96 combined input channels
RPB = 16  # output rows per block
NBLK = H // RPB
SUB_ROWS = 8  # rows per matmul (512 px = one PSUM bank)
SUB_PX = SUB_ROWS * W
# tap order: center (dy=1,dx=1) first so start=True covers every element
TAP_ORDER = [4, 0, 1, 2, 3, 5, 6, 7, 8]

_CACHE: dict = {}


def _build_program():
    import concourse.bacc as bacc
    import concourse.mybir as mybir
    import concourse.tile as tile

    nc = bacc.Bacc("TRN2", target_bir_lowering=False, debug=False)
    f32 = mybir.dt.float32
    bf16 = mybir.dt.bfloat16
    AF = mybir.ActivationFunctionType

    xh_d = nc.dram_tensor("xh", [B_LOC, CTOT, H, W], bf16, kind="ExternalInput").ap()
    c_d = nc.dram_tensor("c", [B_LOC, HC, H, W], bf16, kind="ExternalInput").ap()
    w_d = nc.dram_tensor("w", [CTOT, 9 * 4 * HC], bf16, kind="ExternalInput").ap()
    b_d = nc.dram_tensor("bias", [128, 2], f32, kind="ExternalInput").ap()
    # out[:, 0] = h_new, out[:, 1] = c_new
    out_d = nc.dram_tensor(
        "out", [B_LOC, 2, HC, H, W], bf16, kind="ExternalOutput"
    ).ap()

    with tile.TileContext(nc) as tc:
        with (
            tc.tile_pool(name="const", bufs=1) as constp,
            tc.tile_pool(name="pt", bufs=6) as ptp,
            tc.tile_pool(name="psum0", bufs=2, space="PSUM") as pp0,
            tc.tile_pool(name="psum1", bufs=2, space="PSUM") as pp1,
            tc.tile_pool(name="work", bufs=3) as sp,
        ):
            w_sb = constp.tile([CTOT, 9 * 4 * HC], bf16)
            nc.scalar.dma_start(w_sb[:], w_d)
            b_sb = constp.tile([128, 2], f32)
            nc.scalar.dma_start(b_sb[:], b_d)

            def stage_b(st):
                # deferred tail of a block: tanh(c_new), h_new, output DMA
                b_, y0_, rpb_, so_, chn_, i_ = st
                tch = sp.tile([64, rpb_ * W], bf16, tag="tch", name=f"tch{i_}")
                nc.scalar.activation(tch[:], chn_[64:128, :], AF.Tanh)
                nc.vector.tensor_mul(chn_[0:64, :], so_[:], tch[:])
                nc.sync.dma_start(
                    out_d[b_, :, :, y0_ : y0_ + rpb_, :].rearrange(
                        "t c y x -> (t c) y x"
                    ),
                    chn_[:].rearrange("p (y x) -> p y x", x=W),
                )

            # PE prewarm: ~12 dummy matmuls on zeroed tiles so the HAM
            # clock gate opens before the first real matmul arrives
            dw = constp.tile([CTOT, 128], bf16)
            nc.vector.memset(dw[:], 0.0)
            drh = constp.tile([CTOT, SUB_PX], bf16)
            nc.vector.memset(drh[:], 0.0)
            pwp = pp0.tile([128, RPB * W], f32, tag="P0", name="pw")
            for _ in range(28):
                nc.tensor.matmul(
                    pwp[:, 0:SUB_PX], dw[:], drh[:], start=True, stop=True
                )

            pending = None
            bi = 0
            for b in range(B_LOC):
                rpb = 8 if b == B_LOC - 1 else RPB
                nblk = H // rpb
                for blk in range(nblk):
                    y0 = blk * rpb
                    nrows = rpb + 2
                    blk_px = rpb * W
                    # contiguous input tile; local row L = image row y0-1+L
                    pt = ptp.tile(
                        [CTOT, nrows * W], bf16, tag="ptb", name=f"ptb{bi}"
                    )
                    pt3 = pt[:].rearrange("c (y x) -> c y x", x=W)
                    gs = max(0, y0 - 1)
                    ge = min(H, y0 + rpb + 1)
                    ls = gs - (y0 - 1)
                    le = ge - (y0 - 1)
                    nc.sync.dma_start(pt3[:, ls:le, :], xh_d[b, :, gs:ge, :])

                    P0 = pp0.tile([128, blk_px], f32, tag="P0", name=f"P0_{bi}")
                    P1 = pp1.tile([128, blk_px], f32, tag="P1", name=f"P1_{bi}")
                    for chunk, P in ((0, P0), (1, P1)):
                        P3 = P[:].rearrange("c (y x) -> c y x", x=W)
                        for ti, off in enumerate(TAP_ORDER):
                            dy, dx = off // 3, off % 3
                            lo = off * 256 + chunk * 128
                            lhsT = w_sb[:, lo : lo + 128]
                            # border-valid output ranges for this tap
                            r_lo = 1 if (blk == 0 and dy == 0) else 0
                            r_hi = (
                                rpb - 2
                                if (blk == nblk - 1 and dy == 2)
                                else rpb - 1
                            )
                            cout0, ncols = ((1, 63), (0, 64), (0, 63))[dx]
                            cin0 = cout0 + dx - 1
                            for sub in range(rpb // SUB_ROWS):
                                r0 = max(sub * SUB_ROWS, r_lo)
                                r1 = min(sub * SUB_ROWS + SUB_ROWS - 1, r_hi)
                                rhs = pt3[
                                    :, r0 + dy : r1 + 1 + dy, cin0 : cin0 + ncols
                                ]
                                nc.tensor.matmul(
                                    P3[:, r0 : r1 + 1, cout0 : cout0 + ncols],
                                    lhsT,
                                    rhs,
                                    start=(ti == 0),
                                    stop=(ti == 8),
                                )

                    # elementwise LSTM math for this block
                    # P0 = [f | i], P1 = [o | g] (by 64-partition halves)
                    s_fi = sp.tile([128, blk_px], bf16, tag="sfi", name=f"sfi{bi}")
                    nc.scalar.activation(
                        s_fi[:], P0[:], AF.Sigmoid, bias=b_sb[:, 0:1]
                    )
                    so = sp.tile([64, blk_px], bf16, tag="so", name=f"so{bi}")
                    nc.scalar.activation(
                        so[:], P1[0:64, :], AF.Sigmoid, bias=b_sb[0:64, 1:2]
                    )
                    cg = sp.tile([128, blk_px], bf16, tag="cg", name=f"cg{bi}")
                    nc.scalar.activation(
                        cg[64:128, :], P1[64:128, :], AF.Tanh, bias=b_sb[64:128, 1:2]
                    )
                    nc.gpsimd.dma_start(
                        cg[0:64, :].rearrange("c (y x) -> c y x", x=W),
                        c_d[b, :, y0 : y0 + rpb, :],
                    )
                    # prd = [f*c | i*g]
                    prd = sp.tile([128, blk_px], bf16, tag="prd", name=f"prd{bi}")
                    nc.vector.tensor_mul(prd[:], s_fi[:], cg[:])
                    igc = sp.tile([64, blk_px], bf16, tag="igc", name=f"igc{bi}")
                    nc.vector.tensor_copy(igc[:], prd[64:128, :])
                    # chn = [h_new | c_new] merged output tile
                    chn = sp.tile([128, blk_px], bf16, tag="chn", name=f"chn{bi}")
                    nc.vector.tensor_add(chn[64:128, :], prd[0:64, :], igc[:])
                    if pending is not None:
                        stage_b(pending)
                    pending = (b, y0, rpb, so, chn, bi)
                    bi += 1
            stage_b(pending)

    nc.compile()
    return nc


def get_program():
    if "nc" not in _CACHE:
        _CACHE["nc"] = _build_program()
    return _CACHE["nc"]


def _prep_host(inputs):
    """Pack weights/biases; convert x/h to bf16; build per-core input maps."""
    x = np.asarray(inputs["x"], np.float32)
    h = np.asarray(inputs["hidden_state"], np.float32)
    c = np.ascontiguousarray(np.asarray(inputs["cell_state"], np.float32)).astype(BF16)

    # gate column order [f, i, o, g] -> chunk0=[f,i], chunk1=[o,g]
    gx = [inputs["w_xf"], inputs["w_xi"], inputs["w_xo"], inputs["w_xg"]]
    gh = [inputs["w_hf"], inputs["w_hi"], inputs["w_ho"], inputs["w_hg"]]
    wx = np.stack([np.asarray(a, np.float32) for a in gx])  # [4, HC, C_IN, 3, 3]
    wh = np.stack([np.asarray(a, np.float32) for a in gh])  # [4, HC, HC, 3, 3]
    # -> [c, dy, dx, gate, o] -> [c, 9, 256]
    wxc = np.transpose(wx, (2, 3, 4, 0, 1)).reshape(C_IN, 9, 4 * HC)
    whc = np.transpose(wh, (2, 3, 4, 0, 1)).reshape(HC, 9, 4 * HC)
    wcat = np.concatenate([wxc, whc], 0).reshape(CTOT, 9 * 4 * HC).astype(BF16)

    bf = np.asarray(inputs["b_xf"], np.float32) + np.asarray(inputs["b_hf"], np.float32)
    bi = np.asarray(inputs["b_xi"], np.float32) + np.asarray(inputs["b_hi"], np.float32)
    bo = np.asarray(inputs["b_xo"], np.float32) + np.asarray(inputs["b_ho"], np.float32)
    bg = np.asarray(inputs["b_xg"], np.float32) + np.asarray(inputs["b_hg"], np.float32)
    bias = np.stack(
        [np.concatenate([bf, bi]), np.concatenate([bo, bg])], axis=1
    ).astype(np.float32)  # [128, 2]

    xh = np.concatenate([x, h], axis=1).astype(BF16)  # [B, 96, H, W]

    in_maps = []
    for i in range(N_CORES):
        s = slice(i * B_LOC, (i + 1) * B_LOC)
        in_maps.append(
            {
                "xh": xh[s],
                "c": c[s],
                "w": wcat,
                "bias": bias,
            }
        )
    return in_maps


def run(inputs, trace=False, trace_kwargs=None):
    from concourse.bass_utils import run_bass_kernel_spmd

    nc = get_program()
    in_maps = _prep_host(inputs)
    res = run_bass_kernel_spmd(
        nc,
        in_maps,
        list(range(N_CORES)),
        trace=trace,
        **(trace_kwargs or {}),
    )
    h_new = np.concatenate(
        [np.asarray(r["out"][:, 0]).astype(np.float32) for r in res.results], 0
    )
    c_new = np.concatenate(
        [np.asarray(r["out"][:, 1]).astype(np.float32) for r in res.results], 0
    )
    return (h_new, c_new), res


def kernel(**inputs):
    (h_new, c_new), _ = run(inputs, trace=False)
    return (h_new, c_new)

